# revision 35
# baseline (speedup 1.0000x reference)
"""Trainium2 Bass kernel for nn_LocalFmoeCatEmbedFeedForward.

Strategy (expert-parallel, 8 cores):
  - Host: router (concat -> logits -> softmax -> top-1 gate) + dispatch.
    Tokens are gathered per expert; each expert's tokens are split across
    2 cores (4 experts x 2 = 8 cores).
  - Device (per core), all matmul operands bf16 (enables Fast Weight Load
    so LDWEIGHTS overlaps MATMUL; fp32 weights disable FWL):
      GEMM1: H^T[m,:] = relu(sum_k W1T[k,m].T @ X^T[k,:])   (gate folded
             into X on the host when w1_bias == 0, the common case)
      GEMM2: Y^T[d,:] = sum_k W2T[k,d].T @ H^T[k,:]          (d-major, so
             the token dim is the moving/free dim and C needs no 128
             alignment)
    GEMM1/GEMM2 interleave per token chunk so the PE stays dense.
  - Dummy warm-up matmuls on a zeroed scratch tile run while the input
    DMAs stream in, so the HAM activity monitor un-throttles the PE
    (1.2 -> 2.4 GHz) before the real work starts.
  - x / y are packed k-major / d-major in DRAM so each chunk moves with a
    single DMA trigger (triggers cost ~600ns on the issuing engine).
    Output DMAs are triggered from the otherwise idle Vector engine.
  - Host: scatter rows back; add w2_bias contribution if nonzero.

Fallback (w1_bias != 0): gate cannot be folded into X, so GEMM2 runs
token-major with the gate applied as a per-partition ACT scale; C is
padded to 128.
"""

import os
import sys

sys.path.insert(0, "/opt/trn_rl_repo")

import numpy as np
import ml_dtypes

import concourse.bacc as bacc
import concourse.tile as tile
from concourse import mybir
from concourse import bass_utils

IDIM, EMBED_DIM, NUM_EXPERTS, HIDDEN = 512, 256, 4, 1024
N_CORES = 8
P = 128

BF16 = ml_dtypes.bfloat16


def _chunks(C):
    """Chunk widths: 256 first (small early DMA), 512s, remainder last
    (short drain tail)."""
    if C <= 512:
        return [C]
    out = [256]  # small first chunk -> early PE start
    rem = C - 512  # minus first and last small chunks
    while rem > 512:
        out.append(512)
        rem -= 512
    if rem:
        out.append(rem)
    out.append(256)  # small last chunk -> short ACT/DMA drain tail
    return out


def _plan(cnt, align=16):
    """Pick (C0, V): every core gets C0 main tokens of its own expert
    plus one overflow slot of up to V tokens of a data-chosen expert
    (second weight set shipped as per-core data).  Minimizes per-core
    work C0 + V; falls back to the plain per-expert split (V=0)."""
    maxs = max((c + 1) // 2 for c in cnt)
    base = max(align, -(-maxs // align) * align)
    best = (base, base, 0)
    for C0 in range(256, base + 1, align):
        r = [max(0, c - 2 * C0) for c in cnt]
        if sum(r) == 0:
            continue
        for V in range(align, 513, align):
            if C0 + V >= best[0]:
                break
            slots = sum(-(-x // V) for x in r)
            if slots <= len(cnt) * 2:
                best = (C0 + V, C0, V)
                break
    return best[1], best[2]


def _build_nc_fast(C: int, V: int):
    """Per-core SPMD program, fast path (gate pre-folded, no w1 bias).

    C: main-section columns (own expert).  V: overflow-section columns
    (second expert whose weights arrive as per-core data); 0 disables
    the overflow section."""
    nc = bacc.Bacc("TRN2", target_bir_lowering=False, debug=False,
                   num_devices=N_CORES)
    f32 = mybir.dt.float32
    bf16 = mybir.dt.bfloat16

    K1 = IDIM // P        # 4  k-chunks for GEMM1
    M1 = HIDDEN // P      # 8  m-chunks (H feature blocks)
    K2 = HIDDEN // P      # 8  k-chunks for GEMM2
    DM = IDIM // P        # 4  d-blocks of the output

    xp = nc.dram_tensor("xp", [P, K1 * C], bf16, kind="ExternalInput").ap()
    w1p = nc.dram_tensor("w1p", [P, M1 * K1 * P], bf16,
                         kind="ExternalInput").ap()
    w2p = nc.dram_tensor("w2p", [P, K2 * DM * P], bf16,
                         kind="ExternalInput").ap()
    yp = nc.dram_tensor("yp", [P, DM * C], bf16, kind="ExternalOutput").ap()
    if V:
        xo = nc.dram_tensor("xo", [P, K1 * V], bf16,
                            kind="ExternalInput").ap()
        w1o = nc.dram_tensor("w1o", [P, M1 * K1 * P], bf16,
                             kind="ExternalInput").ap()
        w2o = nc.dram_tensor("w2o", [P, K2 * DM * P], bf16,
                             kind="ExternalInput").ap()
        yo_d = nc.dram_tensor("yo", [P, DM * V], bf16,
                              kind="ExternalOutput").ap()

    chunks = _chunks(C)
    # chunk start offsets
    offs = []
    n0 = 0
    for w in chunks:
        offs.append(n0)
        n0 += w

    with tile.TileContext(nc) as tc:
        with (
            tc.tile_pool(name="sb", bufs=1) as sb_pool,
            tc.tile_pool(name="ps", bufs=1, space="PSUM") as ps_pool,
        ):
            xp_r = xp.rearrange("p (k c) -> p k c", c=C)
            yp_r = yp.rearrange("p (d c) -> p d c", c=C)

            # --- input DMAs: w1 m0-m1 blocks, x chunk0, w1 rest, x
            # chunk1, w2, remaining x chunks.  GEMM1 starts as soon as
            # the first two w1 m-blocks + the small x chunk0 land
            # (~0.5 MB), and consumes m-blocks slower than the w1-rest
            # DMA delivers them.  GEMM2 lags GEMM1 by one chunk so w2
            # is off the critical path.
            w1a = sb_pool.tile([P, M1 * K1 * P], bf16, tag="w1a", name="w1a")
            nc.sync.dma_start(w1a[:, 0:2 * K1 * P], w1p[:, 0:2 * K1 * P])

            xt = sb_pool.tile([P, K1 * C], bf16, tag="xt", name="xt")
            xt_r = xt.rearrange("p (k c) -> p k c", c=C)
            w0 = chunks[0]
            nc.sync.dma_start(xt_r[:, :, 0:w0], xp_r[:, :, 0:w0])

            nc.sync.dma_start(w1a[:, 2 * K1 * P:], w1p[:, 2 * K1 * P:])

            c1_end = offs[1] + chunks[1] if len(chunks) > 1 else C
            if c1_end > w0:
                nc.sync.dma_start(xt_r[:, :, w0:c1_end],
                                  xp_r[:, :, w0:c1_end])

            w2a = sb_pool.tile([P, K2 * DM * P], bf16, tag="w2a", name="w2a")
            nc.sync.dma_start(w2a[:], w2p[:])

            # remaining x chunks, one DMA each (completion sem fires per
            # chunk instead of once at the end of a merged transfer)
            for ci in range(2, len(chunks)):
                nc.sync.dma_start(xt_r[:, :, offs[ci]:offs[ci] + chunks[ci]],
                                  xp_r[:, :, offs[ci]:offs[ci] + chunks[ci]])

            # overflow-section inputs: needed only at the end of the
            # kernel, so they queue behind everything else.
            if V:
                w1b = sb_pool.tile([P, M1 * K1 * P], bf16, tag="w1b",
                                   name="w1b")
                nc.sync.dma_start(w1b[:], w1o[:])
                w2b = sb_pool.tile([P, K2 * DM * P], bf16, tag="w2b",
                                   name="w2b")
                nc.sync.dma_start(w2b[:], w2o[:])
                xto = sb_pool.tile([P, K1 * V], bf16, tag="xto", name="xto")
                xto_r = xto.rearrange("p (k c) -> p k c", c=V)
                nc.sync.dma_start(xto_r[:], xo.rearrange(
                    "p (k c) -> p k c", c=V)[:])

            # --- PE warm-up matmuls on a zeroed scratch tile keep the
            # HAM activity window busy while input DMAs stream in; the
            # HAM un-throttle (1.2 -> 2.4 GHz) needs ~3.4us of gap-free
            # PE activity, so fillers also bridge known DMA waits.
            # 512-col dummies: ~213ns warm / ~427ns cold each.
            scr = sb_pool.tile([P, 512], bf16, tag="scr", name="scr")
            nc.gpsimd.memset(scr[:], 0)

            def fill(n):
                for _ in range(n):
                    ps = ps_pool.tile([P, 512], f32, tag="ps2", bufs=3)
                    nc.tensor.matmul(ps[:], scr[:, 0:P], scr[:],
                                     start=True, stop=True,
                                     skip_group_check=True)

            fill(13)  # ~5.6us cold: covers the w1-m0m1 + x0 DMA

            ht = sb_pool.tile([P, K2 * C], bf16, tag="ht", name="ht")
            ht_r = ht.rearrange("p (k c) -> p k c", c=C)
            if V:
                hto = sb_pool.tile([P, K2 * V], bf16, tag="hto", name="hto")
                hto_r = hto.rearrange("p (k c) -> p k c", c=V)

            def gemm1(w1t, xr, hr, n0, w, first=False):
                for m in range(M1):
                    ps = ps_pool.tile([P, 512], f32, tag="ps1", bufs=4)
                    for k in range(K1):
                        nc.tensor.matmul(
                            ps[:, :w],
                            w1t[:, (m * K1 + k) * P:(m * K1 + k + 1) * P],
                            xr[:, k, n0:n0 + w],
                            start=(k == 0),
                            stop=(k == K1 - 1),
                        )
                    nc.scalar.activation(
                        hr[:, m, n0:n0 + w], ps[:, :w],
                        mybir.ActivationFunctionType.Relu,
                    )
                    if first and m <= 1:
                        fill(2)  # shock absorber for w1-rest DMA jitter

            def gemm2(w2t, hr, yr, n0, w, last=False):
                yt = sb_pool.tile([P, DM * 512], bf16, tag="yo", bufs=4)
                for dm in range(DM):
                    ps = ps_pool.tile([P, 512], f32, tag="ps2", bufs=3)
                    for k in range(K2):
                        nc.tensor.matmul(
                            ps[:, :w],
                            w2t[:, (k * DM + dm) * P:(k * DM + dm + 1) * P],
                            hr[:, k, n0:n0 + w],
                            start=(k == 0),
                            stop=(k == K2 - 1),
                        )
                    if last and dm % 2 == 1:
                        # split the final PSUM->SBUF drain across the
                        # idle Vector engine so the copies run pairwise
                        # concurrently with Scalar's.
                        nc.vector.tensor_scalar_mul(
                            yt[:, dm * w:(dm + 1) * w], ps[:, :w], 1.0)
                    else:
                        nc.scalar.activation(
                            yt[:, dm * w:(dm + 1) * w], ps[:, :w],
                            mybir.ActivationFunctionType.Identity,
                        )
                    if last and w > 256:
                        # per-dm DMA from the idle Sync engine: transfers
                        # overlap the remaining ACTs -> short drain tail.
                        nc.sync.dma_start(yr[:, dm, n0:n0 + w],
                                          yt[:, dm * w:(dm + 1) * w])
                if not (last and w > 256):
                    yt_r = yt[:, 0:DM * w].rearrange("p (d c) -> p d c", c=w)
                    eng = nc.sync if last else nc.scalar
                    eng.dma_start(yr[:, :, n0:n0 + w], yt_r)

            # worklist: main chunks, then overflow chunks
            main = [(w1a, xt_r, ht_r, w2a, yp_r, n0, w)
                    for n0, w in zip(offs, chunks)]
            if V:
                yo_r = yo_d.rearrange("p (d c) -> p d c", c=V)
                o0 = 0
                for wv in _chunks(V):
                    main.append((w1b, xto_r, hto_r, w2b, yo_r, o0, wv))
                    o0 += wv

            # software pipeline: G1c0, G1c1, G2c0, G1c2, G2c1, ...
            gemm1(*main[0][:3], main[0][5], main[0][6], first=True)
            fill(8)  # absorber: x chunk-1 DMA/sem jitter
            for ci in range(1, len(main)):
                gemm1(*main[ci][:3], main[ci][5], main[ci][6])
                w1t, xr, hr, w2t, yr, n0, w = main[ci - 1]
                gemm2(w2t, hr, yr, n0, w)
            w1t, xr, hr, w2t, yr, n0, w = main[-1]
            gemm2(w2t, hr, yr, n0, w, last=True)

    nc.compile()
    return nc


def _build_nc_safe(C: int):
    """Fallback program: w1 bias on device, gate applied in GEMM2 epilogue.

    C must be a multiple of 128 (token-major GEMM2 output tiles)."""
    nc = bacc.Bacc("TRN2", target_bir_lowering=False, debug=False,
                   num_devices=N_CORES)
    f32 = mybir.dt.float32
    bf16 = mybir.dt.bfloat16

    K1 = IDIM // P
    M1 = HIDDEN // P
    K2 = HIDDEN // P
    NT = C // P

    xT = nc.dram_tensor("xT", [IDIM, C], bf16, kind="ExternalInput").ap()
    w1p = nc.dram_tensor("w1p", [P, M1 * K1 * P], bf16,
                         kind="ExternalInput").ap()
    w2p = nc.dram_tensor("w2p", [P, K2 * IDIM], bf16,
                         kind="ExternalInput").ap()
    b1 = nc.dram_tensor("b1", [P, M1], f32, kind="ExternalInput").ap()
    gate = nc.dram_tensor("gate", [P, NT], f32, kind="ExternalInput").ap()
    y = nc.dram_tensor("y", [C, IDIM], f32, kind="ExternalOutput").ap()

    chunks = []
    n0 = 0
    while n0 < C:
        w = min(512, C - n0)
        chunks.append((n0, w))
        n0 += w

    with tile.TileContext(nc) as tc:
        with (
            tc.tile_pool(name="sb", bufs=1) as sb_pool,
            tc.tile_pool(name="yo", bufs=4) as yo_pool,
            tc.tile_pool(name="ps1", bufs=4, space="PSUM") as ps1_pool,
            tc.tile_pool(name="ps2", bufs=3, space="PSUM") as ps2_pool,
        ):
            xT_k = xT.rearrange("(k p) c -> k p c", p=P)

            b1_sb = sb_pool.tile([P, M1], f32, tag="b1")
            nc.sync.dma_start(b1_sb[:], b1[:])
            gate_sb = sb_pool.tile([P, NT], f32, tag="gate")
            nc.sync.dma_start(gate_sb[:], gate[:])

            w1a = sb_pool.tile([P, M1 * K1 * P], bf16, tag="w1a", name="w1a")
            nc.sync.dma_start(w1a[:, 0:K1 * P], w1p[:, 0:K1 * P])

            w0 = chunks[0][1]
            xt_sb = [sb_pool.tile([P, C], bf16, tag=f"xt{k}", name=f"xt{k}")
                     for k in range(K1)]
            for k in range(K1):
                nc.sync.dma_start(xt_sb[k][:, 0:w0], xT_k[k][:, 0:w0])

            nc.sync.dma_start(w1a[:, K1 * P:], w1p[:, K1 * P:])

            w2a = sb_pool.tile([P, K2 * IDIM], bf16, tag="w2a", name="w2a")
            nc.sync.dma_start(w2a[:], w2p[:])
            w2_sb = [w2a[:, k * IDIM:(k + 1) * IDIM] for k in range(K2)]

            if C > w0:
                for k in range(K1):
                    nc.sync.dma_start(xt_sb[k][:, w0:C], xT_k[k][:, w0:C])

            ht_sb = [sb_pool.tile([P, C], bf16, tag=f"ht{m}", name=f"ht{m}")
                     for m in range(M1)]

            for (n0, w) in chunks:
                for m in range(M1):
                    ps = ps1_pool.tile([P, 512], f32, tag="ps1")
                    for k in range(K1):
                        nc.tensor.matmul(
                            ps[:, :w],
                            w1a[:, (m * K1 + k) * P:(m * K1 + k + 1) * P],
                            xt_sb[k][:, n0:n0 + w],
                            start=(k == 0),
                            stop=(k == K1 - 1),
                        )
                    nc.scalar.activation(
                        ht_sb[m][:, n0:n0 + w], ps[:, :w],
                        mybir.ActivationFunctionType.Relu,
                        bias=b1_sb[:, m:m + 1],
                    )
                for t in range(n0 // P, (n0 + w) // P):
                    ps = ps2_pool.tile([P, IDIM], f32, tag="ps2")
                    for k in range(K2):
                        nc.tensor.matmul(
                            ps[:],
                            ht_sb[k][:, t * P:(t + 1) * P],
                            w2_sb[k],
                            start=(k == 0),
                            stop=(k == K2 - 1),
                        )
                    yt = yo_pool.tile([P, IDIM], f32, tag="yo")
                    nc.scalar.activation(
                        yt[:], ps[:],
                        mybir.ActivationFunctionType.Identity,
                        scale=gate_sb[:, t:t + 1],
                    )
                    nc.sync.dma_start(y[t * P:(t + 1) * P, :], yt[:])

    nc.compile()
    return nc


def kernel(inputs, embed, router_weights, w1_weight, w1_bias, w2_weight,
           w2_bias, mask):
    inputs = np.asarray(inputs, np.float32)
    embed = np.asarray(embed, np.float32)
    router_weights = np.asarray(router_weights, np.float32)
    w1_weight = np.asarray(w1_weight, np.float32)
    w1_bias = np.asarray(w1_bias, np.float32)
    w2_weight = np.asarray(w2_weight, np.float32)
    w2_bias = np.asarray(w2_bias, np.float32)
    mask_f = np.asarray(mask).astype(np.float32)

    K1, M1, K2, DM = IDIM // P, HIDDEN // P, HIDDEN // P, IDIM // P
    B, T, D = inputs.shape
    N = B * T
    x = inputs.reshape(N, D)

    # ---- host router: softmax top-1 over concat(embed, inputs) ----
    router_in = np.concatenate([embed.reshape(N, EMBED_DIM), x], axis=1)
    logits = router_in @ router_weights
    logits -= logits.max(axis=1, keepdims=True)
    p = np.exp(logits)
    p /= p.sum(axis=1, keepdims=True)
    gate_idx = np.argmax(p, axis=1)
    gate_val = p[np.arange(N), gate_idx] * mask_f.reshape(N)

    te_list = [np.nonzero(gate_idx == e)[0] for e in range(NUM_EXPERTS)]

    fast = not np.any(w1_bias)

    def pack_x(xg_rows, cols):
        xs = np.zeros((cols, D), np.float32)
        xs[: len(xg_rows)] = xg_rows
        xT = np.ascontiguousarray(xs.T).astype(BF16)  # [512, cols]
        return np.ascontiguousarray(
            xT.reshape(K1, P, cols).transpose(1, 0, 2).reshape(P, K1 * cols))

    def pack_w1(e):
        return np.ascontiguousarray(
            w1_weight[e].T.reshape(K1, P, M1, P)
            .transpose(1, 2, 0, 3).reshape(P, M1 * K1 * P)).astype(BF16)

    def pack_w2(e):
        return np.ascontiguousarray(
            w2_weight[e].T.reshape(K2, P, DM, P)
            .transpose(1, 0, 2, 3).reshape(P, K2 * DM * P)).astype(BF16)

    if fast:
        xg = x * gate_val[:, None]
        C, V = _plan([len(t) for t in te_list])
        # main shards: expert e -> cores 2e, 2e+1 (C tokens each);
        # leftovers go to overflow slots of V tokens, one per core.
        main_idx, over = [], []
        for e in range(NUM_EXPERTS):
            t = te_list[e]
            main_idx.append(t[:C])
            main_idx.append(t[C:2 * C])
            r = t[2 * C:]
            while len(r):
                over.append((e, r[:V]))
                r = r[V:]
        assert len(over) <= N_CORES
        while len(over) < N_CORES:
            over.append((0, np.zeros(0, np.int64)))

        nc = _build_nc_fast(C, V)
        in_maps = []
        for c in range(N_CORES):
            e = c // 2
            m = {
                "xp": pack_x(xg[main_idx[c]], C),
                "w1p": pack_w1(e),
                "w2p": pack_w2(e),
            }
            if V:
                oe, oidx = over[c]
                m["xo"] = pack_x(xg[oidx], V)
                m["w1o"] = pack_w1(oe)
                m["w2o"] = pack_w2(oe)
            in_maps.append(m)
    else:
        shard_idx = []
        for e in range(NUM_EXPERTS):
            t = te_list[e]
            h = (len(t) + 1) // 2
            shard_idx.append(t[:h])
            shard_idx.append(t[h:])
        maxs = max(len(s) for s in shard_idx)
        C = max(P, -(-maxs // P) * P)
        nc = _build_nc_safe(C)
        in_maps = []
        for c in range(N_CORES):
            e = c // 2
            idx = shard_idx[c]
            xs = np.zeros((C, D), np.float32)
            xs[: len(idx)] = x[idx]
            m = {
                "xT": np.ascontiguousarray(xs.T).astype(BF16),
                "w1p": pack_w1(e),
                "w2p": np.ascontiguousarray(
                    w2_weight[e].T.reshape(K2, P, IDIM)
                    .transpose(1, 0, 2).reshape(P, K2 * IDIM)).astype(BF16),
                "b1": np.ascontiguousarray(w1_bias[e].reshape(M1, P).T),
            }
            gs = np.zeros(C, np.float32)
            gs[: len(idx)] = gate_val[idx]
            m["gate"] = np.ascontiguousarray(gs.reshape(C // P, P).T)
            in_maps.append(m)

    trace = bool(os.environ.get("KERNEL_TRACE"))
    kw = {}
    if trace:
        bass_utils.upload_artifacts = lambda tmpdir: f"local:{tmpdir}"
        kw = dict(trace=True, trace_cores=list(range(N_CORES)),
                  tmpdir=os.environ.get("KERNEL_TRACE_DIR") or None)
    try:
        res = bass_utils.run_bass_kernel_spmd(
            nc, in_maps, core_ids=list(range(N_CORES)), **kw)
    except Exception:
        res = bass_utils.run_bass_kernel_spmd(
            nc, in_maps, core_ids=list(range(N_CORES)), **kw)
    if trace:
        kernel.exec_time_ns = res.exec_time_ns
        kernel.mean_exec_time_ns = res.mean_exec_time_ns

    def unpack_y(arr, cols, idx):
        yT = (arr.reshape(P, DM, cols).transpose(1, 0, 2).reshape(IDIM, cols))
        return yT[:, : len(idx)].T.astype(np.float32)

    out = np.zeros((N, D), np.float32)
    for c in range(N_CORES):
        if fast:
            idx = main_idx[c]
            out[idx] = unpack_y(res.results[c]["yp"], C, idx)
            if V:
                oe, oidx = over[c]
                if len(oidx):
                    out[oidx] = unpack_y(res.results[c]["yo"], V, oidx)
        else:
            idx = shard_idx[c]
            out[idx] = res.results[c]["y"][: len(idx)]
    if np.any(w2_bias):
        out += (w2_bias[gate_idx] * gate_val[:, None])
    return out.reshape(B, T, D)


# revision 36
# speedup vs baseline: 1.0108x; 1.0108x over previous
"""Trainium2 Bass kernel for nn_LocalFmoeCatEmbedFeedForward.

Strategy (expert-parallel, 8 cores):
  - Host: router (concat -> logits -> softmax -> top-1 gate) + dispatch.
    Tokens are gathered per expert; each expert's tokens are split across
    2 cores (4 experts x 2 = 8 cores).
  - Device (per core), all matmul operands bf16 (enables Fast Weight Load
    so LDWEIGHTS overlaps MATMUL; fp32 weights disable FWL):
      GEMM1: H^T[m,:] = relu(sum_k W1T[k,m].T @ X^T[k,:])   (gate folded
             into X on the host when w1_bias == 0, the common case)
      GEMM2: Y^T[d,:] = sum_k W2T[k,d].T @ H^T[k,:]          (d-major, so
             the token dim is the moving/free dim and C needs no 128
             alignment)
    GEMM1/GEMM2 interleave per token chunk so the PE stays dense.
  - Dummy warm-up matmuls on a zeroed scratch tile run while the input
    DMAs stream in, so the HAM activity monitor un-throttles the PE
    (1.2 -> 2.4 GHz) before the real work starts.
  - x / y are packed k-major / d-major in DRAM so each chunk moves with a
    single DMA trigger (triggers cost ~600ns on the issuing engine).
    Output DMAs are triggered from the otherwise idle Vector engine.
  - Host: scatter rows back; add w2_bias contribution if nonzero.

Fallback (w1_bias != 0): gate cannot be folded into X, so GEMM2 runs
token-major with the gate applied as a per-partition ACT scale; C is
padded to 128.
"""

import os
import sys

sys.path.insert(0, "/opt/trn_rl_repo")

import numpy as np
import ml_dtypes

import concourse.bacc as bacc
import concourse.tile as tile
from concourse import mybir
from concourse import bass_utils

IDIM, EMBED_DIM, NUM_EXPERTS, HIDDEN = 512, 256, 4, 1024
N_CORES = 8
P = 128

BF16 = ml_dtypes.bfloat16


def _chunks(C):
    """Chunk widths: 256 first (small early DMA), 512s, remainder last
    (short drain tail)."""
    if C <= 512:
        return [C]
    out = [256]  # small first chunk -> early PE start
    rem = C - 512  # minus first and last small chunks
    while rem > 512:
        out.append(512)
        rem -= 512
    if rem:
        out.append(rem)
    out.append(256)  # small last chunk -> short ACT/DMA drain tail
    return out


def _plan(cnt, align=16):
    """Pick (C0, V): every core gets C0 main tokens of its own expert
    plus one overflow slot of up to V tokens of a data-chosen expert
    (second weight set shipped as per-core data).  Minimizes per-core
    work C0 + V; falls back to the plain per-expert split (V=0)."""
    maxs = max((c + 1) // 2 for c in cnt)
    base = max(align, -(-maxs // align) * align)
    best = (base, base, 0)
    for C0 in range(256, base + 1, align):
        r = [max(0, c - 2 * C0) for c in cnt]
        if sum(r) == 0:
            continue
        for V in range(align, 513, align):
            if C0 + V >= best[0]:
                break
            slots = sum(-(-x // V) for x in r)
            if slots <= len(cnt) * 2:
                best = (C0 + V, C0, V)
                break
    return best[1], best[2]


def _build_nc_fast(C: int, V: int):
    """Per-core SPMD program, fast path (gate pre-folded, no w1 bias).

    C: main-section columns (own expert).  V: overflow-section columns
    (second expert whose weights arrive as per-core data); 0 disables
    the overflow section."""
    nc = bacc.Bacc("TRN2", target_bir_lowering=False, debug=False,
                   num_devices=N_CORES)
    f32 = mybir.dt.float32
    bf16 = mybir.dt.bfloat16

    K1 = IDIM // P        # 4  k-chunks for GEMM1
    M1 = HIDDEN // P      # 8  m-chunks (H feature blocks)
    K2 = HIDDEN // P      # 8  k-chunks for GEMM2
    DM = IDIM // P        # 4  d-blocks of the output

    xp = nc.dram_tensor("xp", [P, K1 * C], bf16, kind="ExternalInput").ap()
    w1p = nc.dram_tensor("w1p", [P, M1 * K1 * P], bf16,
                         kind="ExternalInput").ap()
    w2p = nc.dram_tensor("w2p", [P, K2 * DM * P], bf16,
                         kind="ExternalInput").ap()
    yp = nc.dram_tensor("yp", [P, DM * C], bf16, kind="ExternalOutput").ap()
    if V:
        xo = nc.dram_tensor("xo", [P, K1 * V], bf16,
                            kind="ExternalInput").ap()
        w1o = nc.dram_tensor("w1o", [P, M1 * K1 * P], bf16,
                             kind="ExternalInput").ap()
        w2o = nc.dram_tensor("w2o", [P, K2 * DM * P], bf16,
                             kind="ExternalInput").ap()
        yo_d = nc.dram_tensor("yo", [P, DM * V], bf16,
                              kind="ExternalOutput").ap()

    chunks = _chunks(C)
    # chunk start offsets
    offs = []
    n0 = 0
    for w in chunks:
        offs.append(n0)
        n0 += w

    with tile.TileContext(nc) as tc:
        with (
            tc.tile_pool(name="sb", bufs=1) as sb_pool,
            tc.tile_pool(name="ps", bufs=1, space="PSUM") as ps_pool,
        ):
            xp_r = xp.rearrange("p (k c) -> p k c", c=C)
            yp_r = yp.rearrange("p (d c) -> p d c", c=C)

            # --- input DMAs: w1 m0-m1 blocks, x chunk0, w1 rest, x
            # chunk1, w2, remaining x chunks.  GEMM1 starts as soon as
            # the first two w1 m-blocks + the small x chunk0 land
            # (~0.5 MB), and consumes m-blocks slower than the w1-rest
            # DMA delivers them.  GEMM2 lags GEMM1 by one chunk so w2
            # is off the critical path.
            w1a = sb_pool.tile([P, M1 * K1 * P], bf16, tag="w1a", name="w1a")
            nc.sync.dma_start(w1a[:, 0:2 * K1 * P], w1p[:, 0:2 * K1 * P])

            xt = sb_pool.tile([P, K1 * C], bf16, tag="xt", name="xt")
            xt_r = xt.rearrange("p (k c) -> p k c", c=C)
            w0 = chunks[0]
            nc.sync.dma_start(xt_r[:, :, 0:w0], xp_r[:, :, 0:w0])

            nc.sync.dma_start(w1a[:, 2 * K1 * P:], w1p[:, 2 * K1 * P:])

            c1_end = offs[1] + chunks[1] if len(chunks) > 1 else C
            if c1_end > w0:
                nc.sync.dma_start(xt_r[:, :, w0:c1_end],
                                  xp_r[:, :, w0:c1_end])

            w2a = sb_pool.tile([P, K2 * DM * P], bf16, tag="w2a", name="w2a")
            nc.sync.dma_start(w2a[:], w2p[:])

            # remaining x chunks, one DMA each (completion sem fires per
            # chunk instead of once at the end of a merged transfer)
            for ci in range(2, len(chunks)):
                nc.sync.dma_start(xt_r[:, :, offs[ci]:offs[ci] + chunks[ci]],
                                  xp_r[:, :, offs[ci]:offs[ci] + chunks[ci]])

            # overflow-section inputs: needed only at the end of the
            # kernel, so they queue behind everything else.
            if V:
                w1b = sb_pool.tile([P, M1 * K1 * P], bf16, tag="w1b",
                                   name="w1b")
                nc.sync.dma_start(w1b[:], w1o[:])
                w2b = sb_pool.tile([P, K2 * DM * P], bf16, tag="w2b",
                                   name="w2b")
                nc.sync.dma_start(w2b[:], w2o[:])
                xto = sb_pool.tile([P, K1 * V], bf16, tag="xto", name="xto")
                xto_r = xto.rearrange("p (k c) -> p k c", c=V)
                nc.sync.dma_start(xto_r[:], xo.rearrange(
                    "p (k c) -> p k c", c=V)[:])

            # --- PE warm-up matmuls on a zeroed scratch tile keep the
            # HAM activity window busy while input DMAs stream in; the
            # HAM un-throttle (1.2 -> 2.4 GHz) needs ~3.4us of gap-free
            # PE activity, so fillers also bridge known DMA waits.
            # 512-col dummies: ~213ns warm / ~427ns cold each.
            scr = sb_pool.tile([P, 512], bf16, tag="scr", name="scr")
            nc.gpsimd.memset(scr[:], 0)

            def fill(n):
                for _ in range(n):
                    ps = ps_pool.tile([P, 512], f32, tag="ps2", bufs=4)
                    nc.tensor.matmul(ps[:], scr[:, 0:P], scr[:],
                                     start=True, stop=True,
                                     skip_group_check=True)

            fill(13)  # ~5.6us cold: covers the w1-m0m1 + x0 DMA

            ht = sb_pool.tile([P, K2 * C], bf16, tag="ht", name="ht")
            ht_r = ht.rearrange("p (k c) -> p k c", c=C)
            if V:
                hto = sb_pool.tile([P, K2 * V], bf16, tag="hto", name="hto")
                hto_r = hto.rearrange("p (k c) -> p k c", c=V)

            def gemm1(w1t, xr, hr, n0, w, first=False):
                for m in range(M1):
                    ps = ps_pool.tile([P, 512], f32, tag="ps1", bufs=4)
                    for k in range(K1):
                        nc.tensor.matmul(
                            ps[:, :w],
                            w1t[:, (m * K1 + k) * P:(m * K1 + k + 1) * P],
                            xr[:, k, n0:n0 + w],
                            start=(k == 0),
                            stop=(k == K1 - 1),
                        )
                    nc.scalar.activation(
                        hr[:, m, n0:n0 + w], ps[:, :w],
                        mybir.ActivationFunctionType.Relu,
                    )
                    if first and m <= 1:
                        fill(2)  # shock absorber for w1-rest DMA jitter

            def gemm2(w2t, hr, yr, n0, w, last=False):
                yt = sb_pool.tile([P, DM * 512], bf16, tag="yo", bufs=4)
                for dm in range(DM):
                    ps = ps_pool.tile([P, 512], f32, tag="ps2", bufs=4)
                    for k in range(K2):
                        nc.tensor.matmul(
                            ps[:, :w],
                            w2t[:, (k * DM + dm) * P:(k * DM + dm + 1) * P],
                            hr[:, k, n0:n0 + w],
                            start=(k == 0),
                            stop=(k == K2 - 1),
                        )
                    if last and dm % 2 == 1:
                        # split the final PSUM->SBUF drain across the
                        # idle Vector engine so the copies run pairwise
                        # concurrently with Scalar's.
                        nc.vector.tensor_scalar_mul(
                            yt[:, dm * w:(dm + 1) * w], ps[:, :w], 1.0)
                    else:
                        nc.scalar.activation(
                            yt[:, dm * w:(dm + 1) * w], ps[:, :w],
                            mybir.ActivationFunctionType.Identity,
                        )
                    if last and w > 256:
                        # per-dm DMA from the idle Sync engine: transfers
                        # overlap the remaining ACTs -> short drain tail.
                        nc.sync.dma_start(yr[:, dm, n0:n0 + w],
                                          yt[:, dm * w:(dm + 1) * w])
                if not (last and w > 256):
                    yt_r = yt[:, 0:DM * w].rearrange("p (d c) -> p d c", c=w)
                    eng = nc.sync if last else nc.scalar
                    eng.dma_start(yr[:, :, n0:n0 + w], yt_r)

            # worklist: main chunks, then overflow chunks
            main = [(w1a, xt_r, ht_r, w2a, yp_r, n0, w)
                    for n0, w in zip(offs, chunks)]
            if V:
                yo_r = yo_d.rearrange("p (d c) -> p d c", c=V)
                o0 = 0
                for wv in _chunks(V):
                    main.append((w1b, xto_r, hto_r, w2b, yo_r, o0, wv))
                    o0 += wv

            # software pipeline: G1c0, G1c1, G2c0, G1c2, G2c1, ...
            gemm1(*main[0][:3], main[0][5], main[0][6], first=True)
            fill(8)  # absorber: x chunk-1 DMA/sem jitter
            for ci in range(1, len(main)):
                gemm1(*main[ci][:3], main[ci][5], main[ci][6])
                w1t, xr, hr, w2t, yr, n0, w = main[ci - 1]
                gemm2(w2t, hr, yr, n0, w)
            w1t, xr, hr, w2t, yr, n0, w = main[-1]
            gemm2(w2t, hr, yr, n0, w, last=True)

    nc.compile()
    return nc


def _build_nc_safe(C: int):
    """Fallback program: w1 bias on device, gate applied in GEMM2 epilogue.

    C must be a multiple of 128 (token-major GEMM2 output tiles)."""
    nc = bacc.Bacc("TRN2", target_bir_lowering=False, debug=False,
                   num_devices=N_CORES)
    f32 = mybir.dt.float32
    bf16 = mybir.dt.bfloat16

    K1 = IDIM // P
    M1 = HIDDEN // P
    K2 = HIDDEN // P
    NT = C // P

    xT = nc.dram_tensor("xT", [IDIM, C], bf16, kind="ExternalInput").ap()
    w1p = nc.dram_tensor("w1p", [P, M1 * K1 * P], bf16,
                         kind="ExternalInput").ap()
    w2p = nc.dram_tensor("w2p", [P, K2 * IDIM], bf16,
                         kind="ExternalInput").ap()
    b1 = nc.dram_tensor("b1", [P, M1], f32, kind="ExternalInput").ap()
    gate = nc.dram_tensor("gate", [P, NT], f32, kind="ExternalInput").ap()
    y = nc.dram_tensor("y", [C, IDIM], f32, kind="ExternalOutput").ap()

    chunks = []
    n0 = 0
    while n0 < C:
        w = min(512, C - n0)
        chunks.append((n0, w))
        n0 += w

    with tile.TileContext(nc) as tc:
        with (
            tc.tile_pool(name="sb", bufs=1) as sb_pool,
            tc.tile_pool(name="yo", bufs=4) as yo_pool,
            tc.tile_pool(name="ps1", bufs=4, space="PSUM") as ps1_pool,
            tc.tile_pool(name="ps2", bufs=3, space="PSUM") as ps2_pool,
        ):
            xT_k = xT.rearrange("(k p) c -> k p c", p=P)

            b1_sb = sb_pool.tile([P, M1], f32, tag="b1")
            nc.sync.dma_start(b1_sb[:], b1[:])
            gate_sb = sb_pool.tile([P, NT], f32, tag="gate")
            nc.sync.dma_start(gate_sb[:], gate[:])

            w1a = sb_pool.tile([P, M1 * K1 * P], bf16, tag="w1a", name="w1a")
            nc.sync.dma_start(w1a[:, 0:K1 * P], w1p[:, 0:K1 * P])

            w0 = chunks[0][1]
            xt_sb = [sb_pool.tile([P, C], bf16, tag=f"xt{k}", name=f"xt{k}")
                     for k in range(K1)]
            for k in range(K1):
                nc.sync.dma_start(xt_sb[k][:, 0:w0], xT_k[k][:, 0:w0])

            nc.sync.dma_start(w1a[:, K1 * P:], w1p[:, K1 * P:])

            w2a = sb_pool.tile([P, K2 * IDIM], bf16, tag="w2a", name="w2a")
            nc.sync.dma_start(w2a[:], w2p[:])
            w2_sb = [w2a[:, k * IDIM:(k + 1) * IDIM] for k in range(K2)]

            if C > w0:
                for k in range(K1):
                    nc.sync.dma_start(xt_sb[k][:, w0:C], xT_k[k][:, w0:C])

            ht_sb = [sb_pool.tile([P, C], bf16, tag=f"ht{m}", name=f"ht{m}")
                     for m in range(M1)]

            for (n0, w) in chunks:
                for m in range(M1):
                    ps = ps1_pool.tile([P, 512], f32, tag="ps1")
                    for k in range(K1):
                        nc.tensor.matmul(
                            ps[:, :w],
                            w1a[:, (m * K1 + k) * P:(m * K1 + k + 1) * P],
                            xt_sb[k][:, n0:n0 + w],
                            start=(k == 0),
                            stop=(k == K1 - 1),
                        )
                    nc.scalar.activation(
                        ht_sb[m][:, n0:n0 + w], ps[:, :w],
                        mybir.ActivationFunctionType.Relu,
                        bias=b1_sb[:, m:m + 1],
                    )
                for t in range(n0 // P, (n0 + w) // P):
                    ps = ps2_pool.tile([P, IDIM], f32, tag="ps2")
                    for k in range(K2):
                        nc.tensor.matmul(
                            ps[:],
                            ht_sb[k][:, t * P:(t + 1) * P],
                            w2_sb[k],
                            start=(k == 0),
                            stop=(k == K2 - 1),
                        )
                    yt = yo_pool.tile([P, IDIM], f32, tag="yo")
                    nc.scalar.activation(
                        yt[:], ps[:],
                        mybir.ActivationFunctionType.Identity,
                        scale=gate_sb[:, t:t + 1],
                    )
                    nc.sync.dma_start(y[t * P:(t + 1) * P, :], yt[:])

    nc.compile()
    return nc


def kernel(inputs, embed, router_weights, w1_weight, w1_bias, w2_weight,
           w2_bias, mask):
    inputs = np.asarray(inputs, np.float32)
    embed = np.asarray(embed, np.float32)
    router_weights = np.asarray(router_weights, np.float32)
    w1_weight = np.asarray(w1_weight, np.float32)
    w1_bias = np.asarray(w1_bias, np.float32)
    w2_weight = np.asarray(w2_weight, np.float32)
    w2_bias = np.asarray(w2_bias, np.float32)
    mask_f = np.asarray(mask).astype(np.float32)

    K1, M1, K2, DM = IDIM // P, HIDDEN // P, HIDDEN // P, IDIM // P
    B, T, D = inputs.shape
    N = B * T
    x = inputs.reshape(N, D)

    # ---- host router: softmax top-1 over concat(embed, inputs) ----
    router_in = np.concatenate([embed.reshape(N, EMBED_DIM), x], axis=1)
    logits = router_in @ router_weights
    logits -= logits.max(axis=1, keepdims=True)
    p = np.exp(logits)
    p /= p.sum(axis=1, keepdims=True)
    gate_idx = np.argmax(p, axis=1)
    gate_val = p[np.arange(N), gate_idx] * mask_f.reshape(N)

    te_list = [np.nonzero(gate_idx == e)[0] for e in range(NUM_EXPERTS)]

    fast = not np.any(w1_bias)

    def pack_x(xg_rows, cols):
        xs = np.zeros((cols, D), np.float32)
        xs[: len(xg_rows)] = xg_rows
        xT = np.ascontiguousarray(xs.T).astype(BF16)  # [512, cols]
        return np.ascontiguousarray(
            xT.reshape(K1, P, cols).transpose(1, 0, 2).reshape(P, K1 * cols))

    def pack_w1(e):
        return np.ascontiguousarray(
            w1_weight[e].T.reshape(K1, P, M1, P)
            .transpose(1, 2, 0, 3).reshape(P, M1 * K1 * P)).astype(BF16)

    def pack_w2(e):
        return np.ascontiguousarray(
            w2_weight[e].T.reshape(K2, P, DM, P)
            .transpose(1, 0, 2, 3).reshape(P, K2 * DM * P)).astype(BF16)

    if fast:
        xg = x * gate_val[:, None]
        C, V = _plan([len(t) for t in te_list])
        # main shards: expert e -> cores 2e, 2e+1 (C tokens each);
        # leftovers go to overflow slots of V tokens, one per core.
        main_idx, over = [], []
        for e in range(NUM_EXPERTS):
            t = te_list[e]
            main_idx.append(t[:C])
            main_idx.append(t[C:2 * C])
            r = t[2 * C:]
            while len(r):
                over.append((e, r[:V]))
                r = r[V:]
        assert len(over) <= N_CORES
        while len(over) < N_CORES:
            over.append((0, np.zeros(0, np.int64)))

        nc = _build_nc_fast(C, V)
        in_maps = []
        for c in range(N_CORES):
            e = c // 2
            m = {
                "xp": pack_x(xg[main_idx[c]], C),
                "w1p": pack_w1(e),
                "w2p": pack_w2(e),
            }
            if V:
                oe, oidx = over[c]
                m["xo"] = pack_x(xg[oidx], V)
                m["w1o"] = pack_w1(oe)
                m["w2o"] = pack_w2(oe)
            in_maps.append(m)
    else:
        shard_idx = []
        for e in range(NUM_EXPERTS):
            t = te_list[e]
            h = (len(t) + 1) // 2
            shard_idx.append(t[:h])
            shard_idx.append(t[h:])
        maxs = max(len(s) for s in shard_idx)
        C = max(P, -(-maxs // P) * P)
        nc = _build_nc_safe(C)
        in_maps = []
        for c in range(N_CORES):
            e = c // 2
            idx = shard_idx[c]
            xs = np.zeros((C, D), np.float32)
            xs[: len(idx)] = x[idx]
            m = {
                "xT": np.ascontiguousarray(xs.T).astype(BF16),
                "w1p": pack_w1(e),
                "w2p": np.ascontiguousarray(
                    w2_weight[e].T.reshape(K2, P, IDIM)
                    .transpose(1, 0, 2).reshape(P, K2 * IDIM)).astype(BF16),
                "b1": np.ascontiguousarray(w1_bias[e].reshape(M1, P).T),
            }
            gs = np.zeros(C, np.float32)
            gs[: len(idx)] = gate_val[idx]
            m["gate"] = np.ascontiguousarray(gs.reshape(C // P, P).T)
            in_maps.append(m)

    trace = bool(os.environ.get("KERNEL_TRACE"))
    kw = {}
    if trace:
        bass_utils.upload_artifacts = lambda tmpdir: f"local:{tmpdir}"
        kw = dict(trace=True, trace_cores=list(range(N_CORES)),
                  tmpdir=os.environ.get("KERNEL_TRACE_DIR") or None)
    try:
        res = bass_utils.run_bass_kernel_spmd(
            nc, in_maps, core_ids=list(range(N_CORES)), **kw)
    except Exception:
        res = bass_utils.run_bass_kernel_spmd(
            nc, in_maps, core_ids=list(range(N_CORES)), **kw)
    if trace:
        kernel.exec_time_ns = res.exec_time_ns
        kernel.mean_exec_time_ns = res.mean_exec_time_ns

    def unpack_y(arr, cols, idx):
        yT = (arr.reshape(P, DM, cols).transpose(1, 0, 2).reshape(IDIM, cols))
        return yT[:, : len(idx)].T.astype(np.float32)

    out = np.zeros((N, D), np.float32)
    for c in range(N_CORES):
        if fast:
            idx = main_idx[c]
            out[idx] = unpack_y(res.results[c]["yp"], C, idx)
            if V:
                oe, oidx = over[c]
                if len(oidx):
                    out[oidx] = unpack_y(res.results[c]["yo"], V, oidx)
        else:
            idx = shard_idx[c]
            out[idx] = res.results[c]["y"][: len(idx)]
    if np.any(w2_bias):
        out += (w2_bias[gate_idx] * gate_val[:, None])
    return out.reshape(B, T, D)
